# revision 2
# baseline (speedup 1.0000x reference)
"""Trainium2 Bass kernel v2 for LoFTR-style linear attention.

Math (per batch b = one core; H=8 heads, D=32, E=256, L=8192):
  Q = q @ Wq.T + bq ; K = k @ Wk.T + bk ; V = v @ Wv.T + bv
  Qf = elu(Q)+1 ; Kf = elu(K)+1
  KV_h = Kf_h.T @ V_h / L ; Ksum_h = sum_s Kf_h / L
  z = 1/(Qf_h . Ksum_h)  (eps negligible)
  out = concat_h[ (Qf_h z) @ KV_h ] @ Wm.T
      = sum_c (Qf ⊙ zexp)_c.T-chunks @ W2_c,  W2 = blockdiag(KV) @ Wm.T

v2 strategy vs v1 (176us):
  - All casts/transposes to device layouts happen on HOST (numpy): q,k are
    pre-transposed fp8 in DoubleRow-K layout [128, 2, L]; v fp8 natural with a
    ones column; weights fp8/fp16.  No gpsimd cast loads, no PE transposes of
    activations, no xbar DMA.
  - All 4 big GEMMs (Qproj, Kproj, KV, final) run fp8e4 DoubleRow (K=256 per
    instruction).
  - Feature map f(x)=min(exp(x),1)+relu(x): exp on ACT, relu split ACT/DVE,
    combine as STT (4x mode, all-SBUF 2-byte) on DVE.
  - msg matmul + copies eliminated by folding Wm into KV at the phase
    boundary: W2 = blockdiag-mask(KV_true) @ Wm.T (tiny 256x256 work).
  - z chain: zi matmul (fp16) -> DVE reciprocal -> em8 expand matmul ->
    qfts = Qf*ze (DVE) -> final DR matmul -> fp16 out, upcast on host.
  - fp8 range management: KV scaled by 1/L, zi scaled by 1/32 (in ksbd), W2
    scaled by 128; final ACT copy descales by 2^-12.
"""

import sys

for p in ("/opt/trn_rl_repo", "/opt/trn_rl_repo/concourse"):
    if p not in sys.path:
        sys.path.insert(0, p)

from contextlib import ExitStack

import numpy as np

import concourse.bass as bass
import concourse.tile as tile
from concourse import mybir
from concourse.bass_utils import run_bass_kernel_spmd

F32 = mybir.dt.float32
F16 = mybir.dt.float16
BF16 = mybir.dt.bfloat16
FP8 = mybir.dt.float8e4
AF = mybir.ActivationFunctionType
OP = mybir.AluOpType
DR = mybir.MatmulPerfMode.DoubleRow

B, L, E = 8, 8192, 256
H, D = 8, 32
NCORES = 8

GRP = 512                # rows per quad/group
NG = L // GRP            # 16
TS = 32.0                # zi scale (folded into mh8)
SW2 = 128.0              # W2 scale
OUT_SCALE = 1.0 / (TS * SW2)


def build_nc(fix_waits=True):
    nc = bass.Bass()

    qT_h = nc.declare_dram_parameter("qT8", [NG, 128, 2 * GRP], FP8, isOutput=False)
    kT_h = nc.declare_dram_parameter("kT8", [NG, 128, 2 * GRP], FP8, isOutput=False)
    vx_h = nc.declare_dram_parameter("vx16", [NG, 128, 4 * 257], BF16, isOutput=False)
    wq_h = nc.declare_dram_parameter("wq8", [128, 2, E], FP8, isOutput=False)
    wk_h = nc.declare_dram_parameter("wk8", [128, 2, E], FP8, isOutput=False)
    wv_h = nc.declare_dram_parameter("wv16", [128, 2, E], BF16, isOutput=False)
    wm_h = nc.declare_dram_parameter("wm16", [128, 2, E], BF16, isOutput=False)
    bq_h = nc.declare_dram_parameter("bq2", [128, 2], F32, isOutput=False)
    n1_h = nc.declare_dram_parameter("neg1", [128, 1], F32, isOutput=False)
    bq1_h = nc.declare_dram_parameter("bq1p", [128, 2], F32, isOutput=False)
    bk_h = nc.declare_dram_parameter("bk2", [1, 512], BF16, isOutput=False)
    on_h = nc.declare_dram_parameter("ones1", [1, 128], BF16, isOutput=False)
    bvb_h = nc.declare_dram_parameter("bvb", [128, E], F32, isOutput=False)
    mbd_h = nc.declare_dram_parameter("mbd", [128, 128], BF16, isOutput=False)
    mh8_h = nc.declare_dram_parameter("mh8", [128, 2, 8], BF16, isOutput=False)
    em8_h = nc.declare_dram_parameter("em8r", [128, 2, 128], BF16, isOutput=False)
    id_h = nc.declare_dram_parameter("ident16", [128, 128], BF16, isOutput=False)
    z5_h = nc.declare_dram_parameter("one512", [1, 512], BF16, isOutput=False)
    mk1_h = nc.declare_dram_parameter("mask1", [1, 128], BF16, isOutput=False)
    out_h = nc.declare_dram_parameter("out", [NG, 128, 4 * E], F16, isOutput=True)

    with ExitStack() as ctx:
        tc = ctx.enter_context(tile.TileContext(nc))

        const = ctx.enter_context(tc.tile_pool(name="const", bufs=1))
        inp = ctx.enter_context(tc.tile_pool(name="inp", bufs=3))
        erp = ctx.enter_context(tc.tile_pool(name="er", bufs=3))
        kfp = ctx.enter_context(tc.tile_pool(name="kf", bufs=3))

        ctx_kv = ctx.enter_context(ExitStack())
        ps_kv = ctx_kv.enter_context(tc.tile_pool(name="ps_kv", bufs=1, space="PSUM"))

        # ---- constants (critical-path first, spread across DGE queues) --
        wk = const.tile([128, 2, E], FP8)
        nc.sync.dma_start(wk[:], wk_h[:])
        bk2 = const.tile([1, 512], BF16)
        nc.scalar.dma_start(bk2[:], bk_h[:])
        ones1 = const.tile([1, 128], BF16)
        nc.gpsimd.dma_start(ones1[:], on_h[:])
        wq = const.tile([128, 2, E], FP8)
        nc.gpsimd.dma_start(wq[:], wq_h[:])
        bq2 = const.tile([128, 2], F32)
        nc.scalar.dma_start(bq2[:], bq_h[:])
        neg1 = const.tile([128, 1], F32)
        nc.scalar.dma_start(neg1[:], n1_h[:])
        bq1p = const.tile([128, 2], F32)
        nc.gpsimd.dma_start(bq1p[:], bq1_h[:])
        wv = const.tile([128, 2, E], BF16)
        nc.scalar.dma_start(wv[:], wv_h[:])
        wm = const.tile([128, 2, E], BF16)
        nc.gpsimd.dma_start(wm[:], wm_h[:])
        bvb = const.tile([128, E], F32)
        nc.gpsimd.dma_start(bvb[:], bvb_h[:])
        mbd = const.tile([128, 128], BF16)
        nc.scalar.dma_start(mbd[:], mbd_h[:])
        mh8 = const.tile([128, 2, 8], BF16)
        nc.scalar.dma_start(mh8[:], mh8_h[:])
        em8 = const.tile([128, 2, 128], BF16)
        nc.gpsimd.dma_start(em8[:], em8_h[:])
        ident = const.tile([128, 128], BF16)
        nc.scalar.dma_start(ident[:], id_h[:])
        one512 = const.tile([1, 512], BF16)
        nc.gpsimd.dma_start(one512[:], z5_h[:])
        mask1 = const.tile([1, 128], BF16)
        nc.gpsimd.dma_start(mask1[:], mk1_h[:])

        # Qf resident for all groups: [e_loc, chunk, group, l_in_group]
        qfull = const.tile([128, 2, NG, GRP], BF16, name="qfull")

        # persistent KV_raw accumulators (column 256 = Ksum via ones col)
        kv0 = ps_kv.tile([128, 257], F32, tag="kv0")
        kv1 = ps_kv.tile([128, 257], F32, tag="kv1")
        kvp = (kv0, kv1)

        # ===== phase A ==================================================
        ctx_a = ctx.enter_context(ExitStack())
        ps_k = ctx_a.enter_context(tc.tile_pool(name="ps_k", bufs=2, space="PSUM"))
        ps_q = ctx_a.enter_context(tc.tile_pool(name="ps_q", bufs=2, space="PSUM"))

        def emit_kv(kfn, vq, u):
            for t in range(4):
                first = u == 0 and t == 0
                last = u == NG - 1 and t == 3
                for c in (0, 1):
                    nc.tensor.matmul(
                        kvp[c][:],
                        kfn[:, t, 128 * c : 128 * (c + 1)],
                        vq[:, t, :],
                        start=first, stop=last,
                    )

        prev = None
        for u in range(NG):
            kq = inp.tile([128, 2, GRP], FP8, tag="kq")
            nc.gpsimd.dma_start(kq[:].rearrange("p a b -> p (a b)"), kT_h[u])
            vq = inp.tile([128, 4, 257], BF16, tag="vq")
            nc.gpsimd.dma_start(vq[:].rearrange("p a b -> p (a b)"), vx_h[u])
            qq = inp.tile([128, 2, GRP], FP8, tag="qq")
            nc.gpsimd.dma_start(qq[:].rearrange("p a b -> p (a b)"), qT_h[u])

            # ---- K: projection (DR) + bias(+1) fold -------------------
            k_ps = ps_k.tile([128, 4, E], F32, tag="k")
            for j in (0, 1):
                nc.tensor.matmul(
                    k_ps[:, 2 * j : 2 * j + 2, :].rearrange("p a b -> p (a b)"),
                    ones1[:], bk2[:],
                    start=True, stop=False, skip_group_check=True,
                )
            for t in range(4):
                nc.tensor.matmul(
                    k_ps[:, t, :],
                    kq[:, :, 128 * t : 128 * (t + 1)],
                    wk[:],
                    start=False, stop=True,
                    perf_mode=DR, skip_group_check=True,
                )
            # ---- Q: projection (DR); PE covers the kfn latency --------
            q_pss = []
            for c in (0, 1):
                q_ps = ps_q.tile([128, GRP], F32, tag="q")
                nc.tensor.matmul(
                    q_ps[:], wq[:, :, 128 * c : 128 * (c + 1)], qq[:],
                    start=True, stop=True, perf_mode=DR,
                )
                q_pss.append(q_ps)
            # ---- KV accum for the PREVIOUS quad -----------------------
            if prev is not None:
                emit_kv(*prev, u - 1)

            # ---- K feature map: kfn = min(max(pp,1), exp(pp-1)) -------
            kbf = k_ps[:].rearrange("p a b -> p (a b)")
            e_k = erp.tile([128, 1024], BF16, tag="ek")
            nc.scalar.activation(e_k[:], kbf, AF.Exp, bias=neg1[:, 0:1])
            kfn = kfp.tile([128, 4, E], BF16, tag="kfn")
            nc.vector.scalar_tensor_tensor(
                kfn[:].rearrange("p a b -> p (a b)"), kbf, 1.0, e_k[:],
                OP.max, OP.min,
            )
            prev = (kfn, vq)

            # ---- Q feature map: qf = min(exp(p+bq), 1 + relu(p+bq)) ---
            # c=0 via ACT relu + DVE STT; c=1 via DVE TS + TT (balance)
            for c in (0, 1):
                q_ps = q_pss[c]
                e_q = erp.tile([128, GRP], BF16, tag="eq")
                nc.scalar.activation(
                    e_q[:], q_ps[:], AF.Exp, bias=bq2[:, c : c + 1]
                )
                if c == 0:
                    r_q = erp.tile([128, GRP], BF16, tag="rq")
                    nc.scalar.activation(
                        r_q[:], q_ps[:], AF.Relu, bias=bq2[:, 0:1]
                    )
                    nc.vector.scalar_tensor_tensor(
                        qfull[:, 0, u, :], r_q[:], 1.0, e_q[:],
                        OP.add, OP.min,
                    )
                else:
                    m_q = erp.tile([128, GRP], BF16, tag="mq")
                    nc.vector.tensor_scalar(
                        m_q[:], q_ps[:], bq1p[:, c : c + 1], 1.0, OP.add, OP.max
                    )
                    nc.vector.tensor_tensor(
                        qfull[:, c, u, :], e_q[:], m_q[:], OP.min
                    )

        emit_kv(*prev, NG - 1)
        ctx_a.close()

        # ====== phase boundary + all zi batches (interleaved) ===========
        bnd = ctx.enter_context(tc.tile_pool(name="bnd", bufs=1))
        zp = ctx.enter_context(tc.tile_pool(name="z", bufs=4))

        w2 = bnd.tile([128, 2, E], FP8, name="w2")
        ksbd = bnd.tile([128, 2, 8], BF16, name="ksbd")

        ksum_sb = []
        kvr_sb = []
        for c in (0, 1):
            ks = bnd.tile([128, 1], F32, tag=f"ksum{c}")
            nc.vector.tensor_scalar(ks[:], kvp[c][:, 256:257], 1.0 / L, None, OP.mult)
            ksum_sb.append(ks)
            kr = bnd.tile([128, 256], BF16, tag=f"kvr{c}")
            nc.scalar.mul(kr[:], kvp[c][:, 0:256], 1.0 / L)
            kvr_sb.append(kr)
            nc.vector.tensor_scalar(
                ksbd[:, c, :], mh8[:, c, :], ks[:], None, OP.mult
            )

        ctx_kv.close()
        ctx_bd = ctx.enter_context(ExitStack())
        ps_bd = ctx_bd.enter_context(tc.tile_pool(name="ps_bd", bufs=1, space="PSUM"))
        ps_zi = ctx_bd.enter_context(tc.tile_pool(name="ps_zi", bufs=4, space="PSUM"))

        def emit_zi(gb):
            zi4 = ps_zi.tile([128, GRP], F32, tag="zi4")
            nc.tensor.matmul(
                zi4[:], mask1[:], one512[:], start=True, stop=False,
                skip_group_check=True,
            )
            for j in range(4):
                g = 4 * gb + j
                for c in (0, 1):
                    nc.tensor.matmul(
                        zi4[32 * j : 32 * j + 8, :], ksbd[:, c, :],
                        qfull[:, c, g, :], start=False,
                        stop=(j == 3 and c == 1),
                        tile_position=(0, 32 * j), skip_group_check=True,
                    )
            zln = zp.tile([128, GRP], F32, tag="zln")
            nc.scalar.activation(zln[:], zi4[:], AF.Ln)
            zs4 = zp.tile([128, GRP], BF16, tag="zs4", name=f"zs4_{gb}")
            nc.scalar.activation(zs4[:], zln[:], AF.Exp, scale=-1.0)
            return zs4

        zs_all = [emit_zi(0)]

        # transpose KV_raw/L: kvrT[b][e_loc, d_glob]
        kvrT = [bnd.tile([128, 256], BF16, tag=f"kvrT{b}", name=f"kvrT{b}") for b in (0, 1)]
        for b in (0, 1):
            for c in (0, 1):
                t_ps = ps_bd.tile([128, 128], BF16, tag="tp")
                nc.tensor.transpose(
                    t_ps[:], kvr_sb[c][:, 128 * b : 128 * (b + 1)], ident[:]
                )
                nc.scalar.copy(kvrT[b][:, 128 * c : 128 * (c + 1)], t_ps[:])

        zs_all.append(emit_zi(1))

        kvbds = []
        for c in (0, 1):
            csl = slice(128 * c, 128 * (c + 1))
            kvt_ps = ps_bd.tile([128, E], F32, tag="kvt")
            nc.tensor.matmul(
                kvt_ps[:], kvrT[0][:, csl], wv[:, 0, :], start=True, stop=False
            )
            nc.tensor.matmul(
                kvt_ps[:], kvrT[1][:, csl], wv[:, 1, :], start=False, stop=True
            )
            s_t = bnd.tile([128, 128], BF16, tag=f"s{c}")
            nc.vector.scalar_tensor_tensor(
                s_t[:], bvb[:, csl], ksum_sb[c][:], kvt_ps[:, csl],
                OP.mult, OP.add,
            )
            kvbd = bnd.tile([128, 128], BF16, tag=f"kvbd{c}")
            nc.vector.tensor_tensor(kvbd[:], s_t[:], mbd[:], OP.mult)
            kvbds.append(kvbd)

        zs_all.append(emit_zi(2))

        for c in (0, 1):
            tb_ps = ps_bd.tile([128, 128], BF16, tag="tb")
            nc.tensor.transpose(tb_ps[:], kvbds[c][:], ident[:])
            kvbdT = bnd.tile([128, 128], BF16, tag=f"kvbdT{c}")
            nc.scalar.copy(kvbdT[:], tb_ps[:])
            w2_ps = ps_bd.tile([128, E], F32, tag="w2p")
            nc.tensor.matmul(
                w2_ps[:], kvbdT[:], wm[:, c, :], start=True, stop=True
            )
            nc.scalar.mul(w2[:, c, :], w2_ps[:], SW2)

        zs_all.append(emit_zi(3))

        ctx_bd.close()

        # ================= phase B: ze -> qfts -> out ===================
        qfsp = ctx.enter_context(tc.tile_pool(name="qfs", bufs=3))
        osp = ctx.enter_context(tc.tile_pool(name="osb", bufs=3))
        ps_ze = ctx.enter_context(tc.tile_pool(name="ps_ze", bufs=2, space="PSUM"))
        ps_o = ctx.enter_context(tc.tile_pool(name="ps_o", bufs=2, space="PSUM"))

        def emit_ze(g):
            gb, j = g // 4, g % 4
            ze_ps = ps_ze.tile([128, 2, GRP], F32, tag="ze")
            for c in (0, 1):
                nc.tensor.matmul(
                    ze_ps[:, c, :], em8[32 * j : 32 * j + 8, c, :],
                    zs_all[gb][32 * j : 32 * j + 8, :], start=True, stop=True,
                    tile_position=(32 * j, 0),
                )
            return ze_ps

        zes = [emit_ze(0), emit_ze(1)]
        for g in range(NG):
            qfts = qfsp.tile([128, 2, GRP], FP8, tag="qfts")
            nc.vector.tensor_tensor(
                qfts[:], qfull[:, :, g, :], zes[g % 2][:], OP.mult
            )
            if g < NG - 2:
                zes[g % 2] = emit_ze(g + 2)
            o_ps = ps_o.tile([128, 4, E], F32, tag="o")
            for t in range(4):
                nc.tensor.matmul(
                    o_ps[:, t, :],
                    qfts[:, :, 128 * t : 128 * (t + 1)],
                    w2[:],
                    start=True, stop=True, perf_mode=DR,
                )
            o_sb = osp.tile([128, 4 * E], F16, tag="osb")
            nc.scalar.mul(o_sb[:], o_ps[:].rearrange("p a b -> p (a b)"), OUT_SCALE)
            for h in (0, 1):
                hs = slice(2 * E * h, 2 * E * (h + 1))
                nc.sync.dma_start(out_h[g, :, hs], o_sb[:, hs])

    if fix_waits:
        _fix_waits(nc)
    return nc


_WAIT_EXEMPT = {"InstEventSemaphore", "InstUnconditionalBranch", "InstISA"}


def _fix_waits(nc):
    """TPB ISA structs hold limited sem-wait slots; move excess waits onto
    sequencer EventSemaphore instructions inserted before the instruction."""
    n = 0
    for fn in nc.m.functions:
        for blk in fn.blocks:
            il = blk.instructions
            new = []
            changed = False
            for inst in il:
                tname = type(inst).__name__
                if tname not in _WAIT_EXEMPT:
                    limit = 0 if tname == "InstDmaTransposeAnt" else 1
                    si = inst.sync_info
                    waits = list(si.on_wait) if si is not None and si.on_wait else []
                    if len(waits) > limit:
                        move, keep = waits[: len(waits) - limit], waits[len(waits) - limit :]
                        for w in move:
                            es = mybir.InstEventSemaphore(
                                name=f"wait_fence_{n}", ins=[], outs=[],
                                engine=inst.engine,
                            )
                            es.sync_info = mybir.SyncInfo(on_wait=[w], on_update=[])
                            new.append(es)
                            n += 1
                        inst.sync_info = mybir.SyncInfo(
                            on_wait=keep,
                            on_update=list(si.on_update) if si.on_update else [],
                        )
                        changed = True
                new.append(inst)
            if changed:
                blk.instructions = new


_NC = None


def _get_nc():
    global _NC
    if _NC is None:
        _NC = build_nc()
    return _NC


def _host_consts(inputs):
    import ml_dtypes  # noqa

    fp8 = mybir.dt.np(FP8)
    bf16 = ml_dtypes.bfloat16
    Wq, Wk, Wv, Wm = (np.asarray(inputs[n], np.float32) for n in ("Wq", "Wk", "Wv", "Wm"))
    bq, bk, bv = (np.asarray(inputs[n], np.float32) for n in ("bq", "bk", "bv"))

    def wlayout(W, dt):
        # [p, i, e_out] = W.T[(i*128+p), e_out]
        return np.ascontiguousarray(
            W.T.reshape(2, 128, E).transpose(1, 0, 2)
        ).astype(dt)

    consts = {
        "wq8": wlayout(Wq, fp8),
        "wk8": wlayout(Wk, fp8),
        "wv16": wlayout(Wv, bf16),
        "wm16": wlayout(Wm, bf16),
        "bq2": np.ascontiguousarray(bq.reshape(2, 128).T),
        "bk2": np.ascontiguousarray(np.tile(bk + 1.0, 2)[None, :]).astype(bf16),
        "ones1": np.ones((1, 128), bf16),
        "neg1": np.full((128, 1), -1.0, np.float32),
        "bq1p": np.ascontiguousarray(bq.reshape(2, 128).T) + 1.0,
        "one512": np.ones((1, 512), bf16),
        "mask1": ((np.arange(128)[None, :] % 32) >= 8).astype(bf16),
        "bvb": np.ascontiguousarray(np.broadcast_to(bv, (128, E))),
    }
    p = np.arange(128)
    consts["mbd"] = ((p[:, None] // 32) == (p[None, :] // 32)).astype(bf16)
    mh8 = np.zeros((128, 2, 8), np.float32)
    for c in (0, 1):
        mh8[:, c, :] = (np.arange(8)[None, :] == (4 * c + p[:, None] // 32)) / TS
    consts["mh8"] = mh8.astype(bf16)
    em8r = np.zeros((128, 2, 128), np.float32)
    for c in (0, 1):
        pm = p[:, None] % 32
        em8r[:, c, :] = (pm < 8) & ((p[None, :] // 32) == (pm - 4 * c))
    consts["em8r"] = em8r.astype(bf16)
    consts["ident16"] = np.eye(128, dtype=bf16)
    return consts


def _prep_inputs(q, k, v):
    import ml_dtypes
    """q,k,v: [L, E] fp32 -> qT8/kT8 [NG,128,1024], vx8 [NG,128,1028] fp8."""
    fp8 = mybir.dt.np(FP8)

    def tlayout(x):
        # xT[p, i, l] = x[l, i*128+p]; split l into quads
        xT = x.T.reshape(2, 128, L).transpose(1, 0, 2)  # [p, i, l]
        xq = xT.reshape(128, 2, NG, GRP).transpose(2, 0, 1, 3)  # [u, p, i, l']
        return np.ascontiguousarray(xq.reshape(NG, 128, 2 * GRP)).astype(fp8)

    vx = np.ones((NG, 128, 4, 257), np.float32)
    vn = v.reshape(NG, 4, 128, 256).transpose(0, 2, 1, 3)  # [u, p, t, e]
    vx[:, :, :, 0:256] = vn
    return {
        "qT8": tlayout(q),
        "kT8": tlayout(k),
        "vx16": np.ascontiguousarray(vx.reshape(NG, 128, 4 * 257)).astype(ml_dtypes.bfloat16),
    }


def _make_in_maps(inputs):
    consts = _host_consts(inputs)
    q = np.asarray(inputs["q"], np.float32)
    k = np.asarray(inputs["k"], np.float32)
    v = np.asarray(inputs["v"], np.float32)

    in_maps = []
    for b in range(NCORES):
        m = dict(consts)
        m.update(_prep_inputs(q[b], k[b], v[b]))
        in_maps.append(m)
    return in_maps


def _unpack_out(res_out):
    # [NG, 128, 4*E] fp16 -> [L, E] fp32; row = 512g + 128t + p
    a = np.asarray(res_out).reshape(NG, 128, 4, E).transpose(0, 2, 1, 3)
    return a.reshape(L, E).astype(np.float32)


def kernel(**inputs):
    nc = _get_nc()
    res = run_bass_kernel_spmd(nc, _make_in_maps(inputs), list(range(NCORES)))
    out = np.stack([_unpack_out(res.results[b]["out"]) for b in range(NCORES)])
    return out.astype(np.float32)


def kernel_traced(**inputs):
    nc = _get_nc()
    res = run_bass_kernel_spmd(
        nc, _make_in_maps(inputs), list(range(NCORES)), trace=True
    )
    out = np.stack([_unpack_out(res.results[b]["out"]) for b in range(NCORES)])
    return out.astype(np.float32), res


# revision 3
# speedup vs baseline: 1.0034x; 1.0034x over previous
"""Trainium2 Bass kernel v2 for LoFTR-style linear attention.

Math (per batch b = one core; H=8 heads, D=32, E=256, L=8192):
  Q = q @ Wq.T + bq ; K = k @ Wk.T + bk ; V = v @ Wv.T + bv
  Qf = elu(Q)+1 ; Kf = elu(K)+1
  KV_h = Kf_h.T @ V_h / L ; Ksum_h = sum_s Kf_h / L
  z = 1/(Qf_h . Ksum_h)  (eps negligible)
  out = concat_h[ (Qf_h z) @ KV_h ] @ Wm.T
      = sum_c (Qf ⊙ zexp)_c.T-chunks @ W2_c,  W2 = blockdiag(KV) @ Wm.T

v2 strategy vs v1 (176us):
  - All casts/transposes to device layouts happen on HOST (numpy): q,k are
    pre-transposed fp8 in DoubleRow-K layout [128, 2, L]; v fp8 natural with a
    ones column; weights fp8/fp16.  No gpsimd cast loads, no PE transposes of
    activations, no xbar DMA.
  - All 4 big GEMMs (Qproj, Kproj, KV, final) run fp8e4 DoubleRow (K=256 per
    instruction).
  - Feature map f(x)=min(exp(x),1)+relu(x): exp on ACT, relu split ACT/DVE,
    combine as STT (4x mode, all-SBUF 2-byte) on DVE.
  - msg matmul + copies eliminated by folding Wm into KV at the phase
    boundary: W2 = blockdiag-mask(KV_true) @ Wm.T (tiny 256x256 work).
  - z chain: zi matmul (fp16) -> DVE reciprocal -> em8 expand matmul ->
    qfts = Qf*ze (DVE) -> final DR matmul -> fp16 out, upcast on host.
  - fp8 range management: KV scaled by 1/L, zi scaled by 1/32 (in ksbd), W2
    scaled by 128; final ACT copy descales by 2^-12.
"""

import sys

for p in ("/opt/trn_rl_repo", "/opt/trn_rl_repo/concourse"):
    if p not in sys.path:
        sys.path.insert(0, p)

from contextlib import ExitStack

import numpy as np

import concourse.bass as bass
import concourse.tile as tile
from concourse import mybir
from concourse.bass_utils import run_bass_kernel_spmd

F32 = mybir.dt.float32
F16 = mybir.dt.float16
BF16 = mybir.dt.bfloat16
FP8 = mybir.dt.float8e4
AF = mybir.ActivationFunctionType
OP = mybir.AluOpType
DR = mybir.MatmulPerfMode.DoubleRow

B, L, E = 8, 8192, 256
H, D = 8, 32
NCORES = 8

GRP = 512                # rows per quad/group
NG = L // GRP            # 16
TS = 32.0                # zi scale (folded into mh8)
SW2 = 128.0              # W2 scale
OUT_SCALE = 1.0 / (TS * SW2)


def build_nc(fix_waits=True):
    nc = bass.Bass()

    qT_h = nc.declare_dram_parameter("qT8", [NG, 128, 2 * GRP], FP8, isOutput=False)
    kT_h = nc.declare_dram_parameter("kT8", [NG, 128, 2 * GRP], FP8, isOutput=False)
    vx_h = nc.declare_dram_parameter("vx16", [NG, 128, 4 * 257], BF16, isOutput=False)
    wq_h = nc.declare_dram_parameter("wq8", [128, 2, E], FP8, isOutput=False)
    wk_h = nc.declare_dram_parameter("wk8", [128, 2, E], FP8, isOutput=False)
    wv_h = nc.declare_dram_parameter("wv16", [128, 2, E], BF16, isOutput=False)
    wm_h = nc.declare_dram_parameter("wm16", [128, 2, E], BF16, isOutput=False)
    bq_h = nc.declare_dram_parameter("bq2", [128, 2], F32, isOutput=False)
    n1_h = nc.declare_dram_parameter("neg1", [128, 1], F32, isOutput=False)
    bq1_h = nc.declare_dram_parameter("bq1p", [128, 2], F32, isOutput=False)
    bk_h = nc.declare_dram_parameter("bk2", [1, 512], BF16, isOutput=False)
    on_h = nc.declare_dram_parameter("ones1", [1, 128], BF16, isOutput=False)
    bvb_h = nc.declare_dram_parameter("bvb", [128, E], F32, isOutput=False)
    mbd_h = nc.declare_dram_parameter("mbd", [128, 128], BF16, isOutput=False)
    mh8_h = nc.declare_dram_parameter("mh8", [128, 2, 8], BF16, isOutput=False)
    em8_h = nc.declare_dram_parameter("em8r", [128, 2, 128], BF16, isOutput=False)
    id_h = nc.declare_dram_parameter("ident16", [128, 128], BF16, isOutput=False)
    z5_h = nc.declare_dram_parameter("one512", [1, 512], BF16, isOutput=False)
    mk1_h = nc.declare_dram_parameter("mask1", [1, 128], BF16, isOutput=False)
    out_h = nc.declare_dram_parameter("out", [NG, 128, 4 * E], F16, isOutput=True)

    with ExitStack() as ctx:
        tc = ctx.enter_context(tile.TileContext(nc))

        const = ctx.enter_context(tc.tile_pool(name="const", bufs=1))
        inp = ctx.enter_context(tc.tile_pool(name="inp", bufs=3))
        erp = ctx.enter_context(tc.tile_pool(name="er", bufs=3))
        kfp = ctx.enter_context(tc.tile_pool(name="kf", bufs=3))

        ctx_kv = ctx.enter_context(ExitStack())
        ps_kv = ctx_kv.enter_context(tc.tile_pool(name="ps_kv", bufs=1, space="PSUM"))

        # ---- constants (critical-path first, spread across DGE queues) --
        wk = const.tile([128, 2, E], FP8)
        nc.sync.dma_start(wk[:], wk_h[:])
        bk2 = const.tile([1, 512], BF16)
        nc.scalar.dma_start(bk2[:], bk_h[:])
        ones1 = const.tile([1, 128], BF16)
        nc.gpsimd.dma_start(ones1[:], on_h[:])
        wq = const.tile([128, 2, E], FP8)
        nc.gpsimd.dma_start(wq[:], wq_h[:])
        bq2 = const.tile([128, 2], F32)
        nc.scalar.dma_start(bq2[:], bq_h[:])
        neg1 = const.tile([128, 1], F32)
        nc.scalar.dma_start(neg1[:], n1_h[:])
        bq1p = const.tile([128, 2], F32)
        nc.gpsimd.dma_start(bq1p[:], bq1_h[:])
        wv = const.tile([128, 2, E], BF16)
        nc.scalar.dma_start(wv[:], wv_h[:])
        wm = const.tile([128, 2, E], BF16)
        nc.gpsimd.dma_start(wm[:], wm_h[:])
        bvb = const.tile([128, E], F32)
        nc.gpsimd.dma_start(bvb[:], bvb_h[:])
        mbd = const.tile([128, 128], BF16)
        nc.scalar.dma_start(mbd[:], mbd_h[:])
        mh8 = const.tile([128, 2, 8], BF16)
        nc.scalar.dma_start(mh8[:], mh8_h[:])
        em8 = const.tile([128, 2, 128], BF16)
        nc.gpsimd.dma_start(em8[:], em8_h[:])
        ident = const.tile([128, 128], BF16)
        nc.scalar.dma_start(ident[:], id_h[:])
        one512 = const.tile([1, 512], BF16)
        nc.gpsimd.dma_start(one512[:], z5_h[:])
        mask1 = const.tile([1, 128], BF16)
        nc.gpsimd.dma_start(mask1[:], mk1_h[:])

        # Qf resident for all groups: [e_loc, chunk, group, l_in_group]
        qfull = const.tile([128, 2, NG, GRP], BF16, name="qfull")

        # persistent KV_raw accumulators (column 256 = Ksum via ones col)
        kv0 = ps_kv.tile([128, 257], F32, tag="kv0")
        kv1 = ps_kv.tile([128, 257], F32, tag="kv1")
        kvp = (kv0, kv1)

        # ===== phase A ==================================================
        ctx_a = ctx.enter_context(ExitStack())
        ps_k = ctx_a.enter_context(tc.tile_pool(name="ps_k", bufs=2, space="PSUM"))
        ps_q = ctx_a.enter_context(tc.tile_pool(name="ps_q", bufs=2, space="PSUM"))

        def emit_kv(kfn, vq, u):
            for t in range(4):
                first = u == 0 and t == 0
                last = u == NG - 1 and t == 3
                for c in (0, 1):
                    nc.tensor.matmul(
                        kvp[c][:],
                        kfn[:, t, 128 * c : 128 * (c + 1)],
                        vq[:, t, :],
                        start=first, stop=last,
                    )

        prev = None
        for u in range(NG):
            kq = inp.tile([128, 2, GRP], FP8, tag="kq")
            vq = inp.tile([128, 4, 257], BF16, tag="vq")
            qq = inp.tile([128, 2, GRP], FP8, tag="qq")
            if u == 0:
                # startup: split across SP HWDGE queues for parallel transfer
                for hh in (0, 1):
                    hsl = slice(GRP * hh, GRP * (hh + 1))
                    nc.sync.dma_start(
                        kq[:].rearrange("p a b -> p (a b)")[:, hsl],
                        kT_h[u][:, hsl],
                    )
                for hh in (0, 1):
                    hsl = slice(GRP * hh, GRP * (hh + 1))
                    nc.sync.dma_start(
                        qq[:].rearrange("p a b -> p (a b)")[:, hsl],
                        qT_h[u][:, hsl],
                    )
                nc.sync.dma_start(vq[:].rearrange("p a b -> p (a b)"), vx_h[u])
            else:
                nc.gpsimd.dma_start(kq[:].rearrange("p a b -> p (a b)"), kT_h[u])
                nc.gpsimd.dma_start(vq[:].rearrange("p a b -> p (a b)"), vx_h[u])
                nc.gpsimd.dma_start(qq[:].rearrange("p a b -> p (a b)"), qT_h[u])

            # ---- K: projection (DR) + bias(+1) fold -------------------
            k_ps = ps_k.tile([128, 4, E], F32, tag="k")
            for j in (0, 1):
                nc.tensor.matmul(
                    k_ps[:, 2 * j : 2 * j + 2, :].rearrange("p a b -> p (a b)"),
                    ones1[:], bk2[:],
                    start=True, stop=False, skip_group_check=True,
                )
            for t in range(4):
                nc.tensor.matmul(
                    k_ps[:, t, :],
                    kq[:, :, 128 * t : 128 * (t + 1)],
                    wk[:],
                    start=False, stop=True,
                    perf_mode=DR, skip_group_check=True,
                )
            # ---- Q: projection (DR); PE covers the kfn latency --------
            q_pss = []
            for c in (0, 1):
                q_ps = ps_q.tile([128, GRP], F32, tag="q")
                nc.tensor.matmul(
                    q_ps[:], wq[:, :, 128 * c : 128 * (c + 1)], qq[:],
                    start=True, stop=True, perf_mode=DR,
                )
                q_pss.append(q_ps)
            # ---- KV accum for the PREVIOUS quad -----------------------
            if prev is not None:
                emit_kv(*prev, u - 1)

            # ---- K feature map: kfn = min(max(pp,1), exp(pp-1)) -------
            kbf = k_ps[:].rearrange("p a b -> p (a b)")
            e_k = erp.tile([128, 1024], BF16, tag="ek")
            nc.scalar.activation(e_k[:], kbf, AF.Exp, bias=neg1[:, 0:1])
            kfn = kfp.tile([128, 4, E], BF16, tag="kfn")
            nc.vector.scalar_tensor_tensor(
                kfn[:].rearrange("p a b -> p (a b)"), kbf, 1.0, e_k[:],
                OP.max, OP.min,
            )
            prev = (kfn, vq)

            # ---- Q feature map: qf = min(exp(p+bq), 1 + relu(p+bq)) ---
            # c=0 via ACT relu + DVE STT; c=1 via DVE TS + TT (balance)
            for c in (0, 1):
                q_ps = q_pss[c]
                e_q = erp.tile([128, GRP], BF16, tag="eq")
                nc.scalar.activation(
                    e_q[:], q_ps[:], AF.Exp, bias=bq2[:, c : c + 1]
                )
                if c == 0:
                    r_q = erp.tile([128, GRP], BF16, tag="rq")
                    nc.scalar.activation(
                        r_q[:], q_ps[:], AF.Relu, bias=bq2[:, 0:1]
                    )
                    nc.vector.scalar_tensor_tensor(
                        qfull[:, 0, u, :], r_q[:], 1.0, e_q[:],
                        OP.add, OP.min,
                    )
                else:
                    m_q = erp.tile([128, GRP], BF16, tag="mq")
                    nc.vector.tensor_scalar(
                        m_q[:], q_ps[:], bq1p[:, c : c + 1], 1.0, OP.add, OP.max
                    )
                    nc.vector.tensor_tensor(
                        qfull[:, c, u, :], e_q[:], m_q[:], OP.min
                    )

        emit_kv(*prev, NG - 1)
        ctx_a.close()

        # ====== phase boundary + all zi batches (interleaved) ===========
        bnd = ctx.enter_context(tc.tile_pool(name="bnd", bufs=1))
        zp = ctx.enter_context(tc.tile_pool(name="z", bufs=4))

        w2 = bnd.tile([128, 2, E], FP8, name="w2")
        ksbd = bnd.tile([128, 2, 8], BF16, name="ksbd")

        ksum_sb = []
        kvr_sb = []
        for c in (0, 1):
            ks = bnd.tile([128, 1], F32, tag=f"ksum{c}")
            nc.vector.tensor_scalar(ks[:], kvp[c][:, 256:257], 1.0 / L, None, OP.mult)
            ksum_sb.append(ks)
            kr = bnd.tile([128, 256], BF16, tag=f"kvr{c}")
            nc.scalar.mul(kr[:], kvp[c][:, 0:256], 1.0 / L)
            kvr_sb.append(kr)
            nc.vector.tensor_scalar(
                ksbd[:, c, :], mh8[:, c, :], ks[:], None, OP.mult
            )

        ctx_kv.close()
        ctx_bd = ctx.enter_context(ExitStack())
        ps_bd = ctx_bd.enter_context(tc.tile_pool(name="ps_bd", bufs=1, space="PSUM"))
        ps_zi = ctx_bd.enter_context(tc.tile_pool(name="ps_zi", bufs=4, space="PSUM"))

        def emit_zi(gb):
            zi4 = ps_zi.tile([128, GRP], F32, tag="zi4")
            nc.tensor.matmul(
                zi4[:], mask1[:], one512[:], start=True, stop=False,
                skip_group_check=True,
            )
            for j in range(4):
                g = 4 * gb + j
                for c in (0, 1):
                    nc.tensor.matmul(
                        zi4[32 * j : 32 * j + 8, :], ksbd[:, c, :],
                        qfull[:, c, g, :], start=False,
                        stop=(j == 3 and c == 1),
                        tile_position=(0, 32 * j), skip_group_check=True,
                    )
            zln = zp.tile([128, GRP], F32, tag="zln")
            nc.scalar.activation(zln[:], zi4[:], AF.Ln)
            zs4 = zp.tile([128, GRP], BF16, tag="zs4", name=f"zs4_{gb}")
            nc.scalar.activation(zs4[:], zln[:], AF.Exp, scale=-1.0)
            return zs4

        zs_all = [emit_zi(0)]

        # transpose KV_raw/L: kvrT[b][e_loc, d_glob]
        kvrT = [bnd.tile([128, 256], BF16, tag=f"kvrT{b}", name=f"kvrT{b}") for b in (0, 1)]
        for b in (0, 1):
            for c in (0, 1):
                t_ps = ps_bd.tile([128, 128], BF16, tag="tp")
                nc.tensor.transpose(
                    t_ps[:], kvr_sb[c][:, 128 * b : 128 * (b + 1)], ident[:]
                )
                nc.scalar.copy(kvrT[b][:, 128 * c : 128 * (c + 1)], t_ps[:])

        zs_all.append(emit_zi(1))

        kvbds = []
        for c in (0, 1):
            csl = slice(128 * c, 128 * (c + 1))
            kvt_ps = ps_bd.tile([128, E], F32, tag="kvt")
            nc.tensor.matmul(
                kvt_ps[:], kvrT[0][:, csl], wv[:, 0, :], start=True, stop=False
            )
            nc.tensor.matmul(
                kvt_ps[:], kvrT[1][:, csl], wv[:, 1, :], start=False, stop=True
            )
            s_t = bnd.tile([128, 128], BF16, tag=f"s{c}")
            nc.vector.scalar_tensor_tensor(
                s_t[:], bvb[:, csl], ksum_sb[c][:], kvt_ps[:, csl],
                OP.mult, OP.add,
            )
            kvbd = bnd.tile([128, 128], BF16, tag=f"kvbd{c}")
            nc.vector.tensor_tensor(kvbd[:], s_t[:], mbd[:], OP.mult)
            kvbds.append(kvbd)

        zs_all.append(emit_zi(2))

        for c in (0, 1):
            tb_ps = ps_bd.tile([128, 128], BF16, tag="tb")
            nc.tensor.transpose(tb_ps[:], kvbds[c][:], ident[:])
            kvbdT = bnd.tile([128, 128], BF16, tag=f"kvbdT{c}")
            nc.scalar.copy(kvbdT[:], tb_ps[:])
            w2_ps = ps_bd.tile([128, E], F32, tag="w2p")
            nc.tensor.matmul(
                w2_ps[:], kvbdT[:], wm[:, c, :], start=True, stop=True
            )
            nc.scalar.mul(w2[:, c, :], w2_ps[:], SW2)

        zs_all.append(emit_zi(3))

        ctx_bd.close()

        # ================= phase B: ze -> qfts -> out ===================
        qfsp = ctx.enter_context(tc.tile_pool(name="qfs", bufs=3))
        osp = ctx.enter_context(tc.tile_pool(name="osb", bufs=3))
        ps_ze = ctx.enter_context(tc.tile_pool(name="ps_ze", bufs=2, space="PSUM"))
        ps_o = ctx.enter_context(tc.tile_pool(name="ps_o", bufs=2, space="PSUM"))

        def emit_ze(g):
            gb, j = g // 4, g % 4
            ze_ps = ps_ze.tile([128, 2, GRP], F32, tag="ze")
            for c in (0, 1):
                nc.tensor.matmul(
                    ze_ps[:, c, :], em8[32 * j : 32 * j + 8, c, :],
                    zs_all[gb][32 * j : 32 * j + 8, :], start=True, stop=True,
                    tile_position=(32 * j, 0),
                )
            return ze_ps

        zes = [emit_ze(0), emit_ze(1)]
        for g in range(NG):
            qfts = qfsp.tile([128, 2, GRP], FP8, tag="qfts")
            nc.vector.tensor_tensor(
                qfts[:], qfull[:, :, g, :], zes[g % 2][:], OP.mult
            )
            if g < NG - 2:
                zes[g % 2] = emit_ze(g + 2)
            o_ps = ps_o.tile([128, 4, E], F32, tag="o")
            for t in range(4):
                nc.tensor.matmul(
                    o_ps[:, t, :],
                    qfts[:, :, 128 * t : 128 * (t + 1)],
                    w2[:],
                    start=True, stop=True, perf_mode=DR,
                )
            o_sb = osp.tile([128, 4 * E], F16, tag="osb")
            nc.scalar.mul(o_sb[:], o_ps[:].rearrange("p a b -> p (a b)"), OUT_SCALE)
            for h in (0, 1):
                hs = slice(2 * E * h, 2 * E * (h + 1))
                nc.sync.dma_start(out_h[g, :, hs], o_sb[:, hs])

    if fix_waits:
        _fix_waits(nc)
    return nc


_WAIT_EXEMPT = {"InstEventSemaphore", "InstUnconditionalBranch", "InstISA"}


def _fix_waits(nc):
    """TPB ISA structs hold limited sem-wait slots; move excess waits onto
    sequencer EventSemaphore instructions inserted before the instruction."""
    n = 0
    for fn in nc.m.functions:
        for blk in fn.blocks:
            il = blk.instructions
            new = []
            changed = False
            for inst in il:
                tname = type(inst).__name__
                if tname not in _WAIT_EXEMPT:
                    limit = 0 if tname == "InstDmaTransposeAnt" else 1
                    si = inst.sync_info
                    waits = list(si.on_wait) if si is not None and si.on_wait else []
                    if len(waits) > limit:
                        move, keep = waits[: len(waits) - limit], waits[len(waits) - limit :]
                        for w in move:
                            es = mybir.InstEventSemaphore(
                                name=f"wait_fence_{n}", ins=[], outs=[],
                                engine=inst.engine,
                            )
                            es.sync_info = mybir.SyncInfo(on_wait=[w], on_update=[])
                            new.append(es)
                            n += 1
                        inst.sync_info = mybir.SyncInfo(
                            on_wait=keep,
                            on_update=list(si.on_update) if si.on_update else [],
                        )
                        changed = True
                new.append(inst)
            if changed:
                blk.instructions = new


_NC = None


def _get_nc():
    global _NC
    if _NC is None:
        _NC = build_nc()
    return _NC


def _host_consts(inputs):
    import ml_dtypes  # noqa

    fp8 = mybir.dt.np(FP8)
    bf16 = ml_dtypes.bfloat16
    Wq, Wk, Wv, Wm = (np.asarray(inputs[n], np.float32) for n in ("Wq", "Wk", "Wv", "Wm"))
    bq, bk, bv = (np.asarray(inputs[n], np.float32) for n in ("bq", "bk", "bv"))

    def wlayout(W, dt):
        # [p, i, e_out] = W.T[(i*128+p), e_out]
        return np.ascontiguousarray(
            W.T.reshape(2, 128, E).transpose(1, 0, 2)
        ).astype(dt)

    consts = {
        "wq8": wlayout(Wq, fp8),
        "wk8": wlayout(Wk, fp8),
        "wv16": wlayout(Wv, bf16),
        "wm16": wlayout(Wm, bf16),
        "bq2": np.ascontiguousarray(bq.reshape(2, 128).T),
        "bk2": np.ascontiguousarray(np.tile(bk + 1.0, 2)[None, :]).astype(bf16),
        "ones1": np.ones((1, 128), bf16),
        "neg1": np.full((128, 1), -1.0, np.float32),
        "bq1p": np.ascontiguousarray(bq.reshape(2, 128).T) + 1.0,
        "one512": np.ones((1, 512), bf16),
        "mask1": ((np.arange(128)[None, :] % 32) >= 8).astype(bf16),
        "bvb": np.ascontiguousarray(np.broadcast_to(bv, (128, E))),
    }
    p = np.arange(128)
    consts["mbd"] = ((p[:, None] // 32) == (p[None, :] // 32)).astype(bf16)
    mh8 = np.zeros((128, 2, 8), np.float32)
    for c in (0, 1):
        mh8[:, c, :] = (np.arange(8)[None, :] == (4 * c + p[:, None] // 32)) / TS
    consts["mh8"] = mh8.astype(bf16)
    em8r = np.zeros((128, 2, 128), np.float32)
    for c in (0, 1):
        pm = p[:, None] % 32
        em8r[:, c, :] = (pm < 8) & ((p[None, :] // 32) == (pm - 4 * c))
    consts["em8r"] = em8r.astype(bf16)
    consts["ident16"] = np.eye(128, dtype=bf16)
    return consts


def _prep_inputs(q, k, v):
    import ml_dtypes
    """q,k,v: [L, E] fp32 -> qT8/kT8 [NG,128,1024], vx8 [NG,128,1028] fp8."""
    fp8 = mybir.dt.np(FP8)

    def tlayout(x):
        # xT[p, i, l] = x[l, i*128+p]; split l into quads
        xT = x.T.reshape(2, 128, L).transpose(1, 0, 2)  # [p, i, l]
        xq = xT.reshape(128, 2, NG, GRP).transpose(2, 0, 1, 3)  # [u, p, i, l']
        return np.ascontiguousarray(xq.reshape(NG, 128, 2 * GRP)).astype(fp8)

    vx = np.ones((NG, 128, 4, 257), np.float32)
    vn = v.reshape(NG, 4, 128, 256).transpose(0, 2, 1, 3)  # [u, p, t, e]
    vx[:, :, :, 0:256] = vn
    return {
        "qT8": tlayout(q),
        "kT8": tlayout(k),
        "vx16": np.ascontiguousarray(vx.reshape(NG, 128, 4 * 257)).astype(ml_dtypes.bfloat16),
    }


def _make_in_maps(inputs):
    consts = _host_consts(inputs)
    q = np.asarray(inputs["q"], np.float32)
    k = np.asarray(inputs["k"], np.float32)
    v = np.asarray(inputs["v"], np.float32)

    in_maps = []
    for b in range(NCORES):
        m = dict(consts)
        m.update(_prep_inputs(q[b], k[b], v[b]))
        in_maps.append(m)
    return in_maps


def _unpack_out(res_out):
    # [NG, 128, 4*E] fp16 -> [L, E] fp32; row = 512g + 128t + p
    a = np.asarray(res_out).reshape(NG, 128, 4, E).transpose(0, 2, 1, 3)
    return a.reshape(L, E).astype(np.float32)


def kernel(**inputs):
    nc = _get_nc()
    res = run_bass_kernel_spmd(nc, _make_in_maps(inputs), list(range(NCORES)))
    out = np.stack([_unpack_out(res.results[b]["out"]) for b in range(NCORES)])
    return out.astype(np.float32)


def kernel_traced(**inputs):
    nc = _get_nc()
    res = run_bass_kernel_spmd(
        nc, _make_in_maps(inputs), list(range(NCORES)), trace=True
    )
    out = np.stack([_unpack_out(res.results[b]["out"]) for b in range(NCORES)])
    return out.astype(np.float32), res


# revision 4
# speedup vs baseline: 1.0117x; 1.0082x over previous
"""Trainium2 Bass kernel v2 for LoFTR-style linear attention.

Math (per batch b = one core; H=8 heads, D=32, E=256, L=8192):
  Q = q @ Wq.T + bq ; K = k @ Wk.T + bk ; V = v @ Wv.T + bv
  Qf = elu(Q)+1 ; Kf = elu(K)+1
  KV_h = Kf_h.T @ V_h / L ; Ksum_h = sum_s Kf_h / L
  z = 1/(Qf_h . Ksum_h)  (eps negligible)
  out = concat_h[ (Qf_h z) @ KV_h ] @ Wm.T
      = sum_c (Qf ⊙ zexp)_c.T-chunks @ W2_c,  W2 = blockdiag(KV) @ Wm.T

v2 strategy vs v1 (176us):
  - All casts/transposes to device layouts happen on HOST (numpy): q,k are
    pre-transposed fp8 in DoubleRow-K layout [128, 2, L]; v fp8 natural with a
    ones column; weights fp8/fp16.  No gpsimd cast loads, no PE transposes of
    activations, no xbar DMA.
  - All 4 big GEMMs (Qproj, Kproj, KV, final) run fp8e4 DoubleRow (K=256 per
    instruction).
  - Feature map f(x)=min(exp(x),1)+relu(x): exp on ACT, relu split ACT/DVE,
    combine as STT (4x mode, all-SBUF 2-byte) on DVE.
  - msg matmul + copies eliminated by folding Wm into KV at the phase
    boundary: W2 = blockdiag-mask(KV_true) @ Wm.T (tiny 256x256 work).
  - z chain: zi matmul (fp16) -> DVE reciprocal -> em8 expand matmul ->
    qfts = Qf*ze (DVE) -> final DR matmul -> fp16 out, upcast on host.
  - fp8 range management: KV scaled by 1/L, zi scaled by 1/32 (in ksbd), W2
    scaled by 128; final ACT copy descales by 2^-12.
"""

import sys

for p in ("/opt/trn_rl_repo", "/opt/trn_rl_repo/concourse"):
    if p not in sys.path:
        sys.path.insert(0, p)

from contextlib import ExitStack

import numpy as np

import concourse.bass as bass
import concourse.tile as tile
from concourse import mybir
from concourse.bass_utils import run_bass_kernel_spmd

F32 = mybir.dt.float32
F16 = mybir.dt.float16
BF16 = mybir.dt.bfloat16
FP8 = mybir.dt.float8e4
AF = mybir.ActivationFunctionType
OP = mybir.AluOpType
DR = mybir.MatmulPerfMode.DoubleRow

B, L, E = 8, 8192, 256
H, D = 8, 32
NCORES = 8

GRP = 512                # rows per quad/group
NG = L // GRP            # 16
TS = 32.0                # zi scale (folded into mh8)
SW2 = 128.0              # W2 scale
OUT_SCALE = 1.0 / (TS * SW2)


def build_nc(fix_waits=True):
    nc = bass.Bass()

    qT_h = nc.declare_dram_parameter("qT8", [NG, 128, 2 * GRP], FP8, isOutput=False)
    kT_h = nc.declare_dram_parameter("kT8", [NG, 128, 2 * GRP], FP8, isOutput=False)
    vx_h = nc.declare_dram_parameter("vx16", [NG, 128, 4 * 257], BF16, isOutput=False)
    wq_h = nc.declare_dram_parameter("wq8", [128, 2, E], FP8, isOutput=False)
    wk_h = nc.declare_dram_parameter("wk8", [128, 2, E], FP8, isOutput=False)
    wv_h = nc.declare_dram_parameter("wv16", [128, 2, E], BF16, isOutput=False)
    wm_h = nc.declare_dram_parameter("wm16", [128, 2, E], BF16, isOutput=False)
    bq_h = nc.declare_dram_parameter("bq2", [128, 2], F32, isOutput=False)
    n1_h = nc.declare_dram_parameter("neg1", [128, 1], F32, isOutput=False)
    bq1_h = nc.declare_dram_parameter("bq1p", [128, 2], F32, isOutput=False)
    bk_h = nc.declare_dram_parameter("bk2", [1, 512], BF16, isOutput=False)
    on_h = nc.declare_dram_parameter("ones1", [1, 128], BF16, isOutput=False)
    bvb_h = nc.declare_dram_parameter("bvb", [128, E], F32, isOutput=False)
    mbd_h = nc.declare_dram_parameter("mbd", [128, 128], BF16, isOutput=False)
    mh8_h = nc.declare_dram_parameter("mh8", [128, 2, 8], BF16, isOutput=False)
    em8_h = nc.declare_dram_parameter("em8r", [128, 2, 128], BF16, isOutput=False)
    id_h = nc.declare_dram_parameter("ident16", [128, 128], BF16, isOutput=False)
    z5_h = nc.declare_dram_parameter("one512", [1, 512], BF16, isOutput=False)
    mk1_h = nc.declare_dram_parameter("mask1", [1, 128], BF16, isOutput=False)
    out_h = nc.declare_dram_parameter("out", [NG, 128, 4 * E], F16, isOutput=True)

    with ExitStack() as ctx:
        tc = ctx.enter_context(tile.TileContext(nc))

        const = ctx.enter_context(tc.tile_pool(name="const", bufs=1))
        inp = ctx.enter_context(tc.tile_pool(name="inp", bufs=3))
        erp = ctx.enter_context(tc.tile_pool(name="er", bufs=3))
        kfp = ctx.enter_context(tc.tile_pool(name="kf", bufs=3))

        ctx_kv = ctx.enter_context(ExitStack())
        ps_kv = ctx_kv.enter_context(tc.tile_pool(name="ps_kv", bufs=1, space="PSUM"))

        # ---- constants (critical-path first, spread across DGE queues) --
        wk = const.tile([128, 2, E], FP8)
        nc.scalar.dma_start(wk[:], wk_h[:])
        bk2 = const.tile([1, 512], BF16)
        nc.scalar.dma_start(bk2[:], bk_h[:])
        ones1 = const.tile([1, 128], BF16)
        nc.gpsimd.dma_start(ones1[:], on_h[:])
        wq = const.tile([128, 2, E], FP8)
        nc.gpsimd.dma_start(wq[:], wq_h[:])
        bq2 = const.tile([128, 2], F32)
        nc.scalar.dma_start(bq2[:], bq_h[:])
        neg1 = const.tile([128, 1], F32)
        nc.scalar.dma_start(neg1[:], n1_h[:])
        bq1p = const.tile([128, 2], F32)
        nc.gpsimd.dma_start(bq1p[:], bq1_h[:])
        wv = const.tile([128, 2, E], BF16)
        nc.scalar.dma_start(wv[:], wv_h[:])
        wm = const.tile([128, 2, E], BF16)
        nc.gpsimd.dma_start(wm[:], wm_h[:])
        bvb = const.tile([128, E], F32)
        nc.gpsimd.dma_start(bvb[:], bvb_h[:])
        mbd = const.tile([128, 128], BF16)
        nc.scalar.dma_start(mbd[:], mbd_h[:])
        mh8 = const.tile([128, 2, 8], BF16)
        nc.scalar.dma_start(mh8[:], mh8_h[:])
        em8 = const.tile([128, 2, 128], BF16)
        nc.gpsimd.dma_start(em8[:], em8_h[:])
        ident = const.tile([128, 128], BF16)
        nc.scalar.dma_start(ident[:], id_h[:])
        one512 = const.tile([1, 512], BF16)
        nc.gpsimd.dma_start(one512[:], z5_h[:])
        mask1 = const.tile([1, 128], BF16)
        nc.gpsimd.dma_start(mask1[:], mk1_h[:])

        # Qf resident for all groups: [e_loc, chunk, group, l_in_group]
        qfull = const.tile([128, 2, NG, GRP], BF16, name="qfull")

        # persistent KV_raw accumulators (column 256 = Ksum via ones col)
        kv0 = ps_kv.tile([128, 257], F32, tag="kv0")
        kv1 = ps_kv.tile([128, 257], F32, tag="kv1")
        kvp = (kv0, kv1)

        # ===== phase A ==================================================
        ctx_a = ctx.enter_context(ExitStack())
        ps_k = ctx_a.enter_context(tc.tile_pool(name="ps_k", bufs=2, space="PSUM"))
        ps_q = ctx_a.enter_context(tc.tile_pool(name="ps_q", bufs=2, space="PSUM"))

        def emit_kv(kfn, vq, u):
            for t in range(4):
                first = u == 0 and t == 0
                last = u == NG - 1 and t == 3
                for c in (0, 1):
                    nc.tensor.matmul(
                        kvp[c][:],
                        kfn[:, t, 128 * c : 128 * (c + 1)],
                        vq[:, t, :],
                        start=first, stop=last,
                    )

        prev = None
        for u in range(NG):
            kq = inp.tile([128, 2, GRP], FP8, tag="kq")
            vq = inp.tile([128, 4, 257], BF16, tag="vq")
            qq = inp.tile([128, 2, GRP], FP8, tag="qq")
            if u == 0:
                # startup: split across SP HWDGE queues for parallel transfer
                for hh in (0, 1):
                    hsl = slice(GRP * hh, GRP * (hh + 1))
                    nc.sync.dma_start(
                        kq[:].rearrange("p a b -> p (a b)")[:, hsl],
                        kT_h[u][:, hsl],
                    )
                for hh in (0, 1):
                    hsl = slice(GRP * hh, GRP * (hh + 1))
                    nc.sync.dma_start(
                        qq[:].rearrange("p a b -> p (a b)")[:, hsl],
                        qT_h[u][:, hsl],
                    )
                nc.sync.dma_start(vq[:].rearrange("p a b -> p (a b)"), vx_h[u])
            else:
                nc.gpsimd.dma_start(kq[:].rearrange("p a b -> p (a b)"), kT_h[u])
                nc.gpsimd.dma_start(vq[:].rearrange("p a b -> p (a b)"), vx_h[u])
                nc.gpsimd.dma_start(qq[:].rearrange("p a b -> p (a b)"), qT_h[u])

            # ---- K: projection (DR) + bias(+1) fold -------------------
            k_ps = ps_k.tile([128, 4, E], F32, tag="k")
            for j in (0, 1):
                nc.tensor.matmul(
                    k_ps[:, 2 * j : 2 * j + 2, :].rearrange("p a b -> p (a b)"),
                    ones1[:], bk2[:],
                    start=True, stop=False, skip_group_check=True,
                )
            for t in range(4):
                nc.tensor.matmul(
                    k_ps[:, t, :],
                    kq[:, :, 128 * t : 128 * (t + 1)],
                    wk[:],
                    start=False, stop=True,
                    perf_mode=DR, skip_group_check=True,
                )
            # ---- Q: projection (DR); PE covers the kfn latency --------
            q_pss = []
            for c in (0, 1):
                q_ps = ps_q.tile([128, GRP], F32, tag="q")
                nc.tensor.matmul(
                    q_ps[:], wq[:, :, 128 * c : 128 * (c + 1)], qq[:],
                    start=True, stop=True, perf_mode=DR,
                )
                q_pss.append(q_ps)
            # ---- KV accum for the PREVIOUS quad -----------------------
            if prev is not None:
                emit_kv(*prev, u - 1)

            # ---- K feature map: kfn = min(max(pp,1), exp(pp-1)) -------
            kbf = k_ps[:].rearrange("p a b -> p (a b)")
            e_k = erp.tile([128, 1024], BF16, tag="ek")
            nc.scalar.activation(e_k[:], kbf, AF.Exp, bias=neg1[:, 0:1])
            kfn = kfp.tile([128, 4, E], BF16, tag="kfn")
            nc.vector.scalar_tensor_tensor(
                kfn[:].rearrange("p a b -> p (a b)"), kbf, 1.0, e_k[:],
                OP.max, OP.min,
            )
            prev = (kfn, vq)

            # ---- Q feature map: qf = min(exp(p+bq), 1 + relu(p+bq)) ---
            # c=0 via ACT relu + DVE STT; c=1 via DVE TS + TT (balance)
            for c in (0, 1):
                q_ps = q_pss[c]
                e_q = erp.tile([128, GRP], BF16, tag="eq")
                nc.scalar.activation(
                    e_q[:], q_ps[:], AF.Exp, bias=bq2[:, c : c + 1]
                )
                if c == 0:
                    r_q = erp.tile([128, GRP], BF16, tag="rq")
                    nc.scalar.activation(
                        r_q[:], q_ps[:], AF.Relu, bias=bq2[:, 0:1]
                    )
                    nc.vector.scalar_tensor_tensor(
                        qfull[:, 0, u, :], r_q[:], 1.0, e_q[:],
                        OP.add, OP.min,
                    )
                else:
                    m_q = erp.tile([128, GRP], BF16, tag="mq")
                    nc.vector.tensor_scalar(
                        m_q[:], q_ps[:], bq1p[:, c : c + 1], 1.0, OP.add, OP.max
                    )
                    nc.vector.tensor_tensor(
                        qfull[:, c, u, :], e_q[:], m_q[:], OP.min
                    )

        emit_kv(*prev, NG - 1)
        ctx_a.close()

        # ====== phase boundary + all zi batches (interleaved) ===========
        bnd = ctx.enter_context(tc.tile_pool(name="bnd", bufs=1))
        zp = ctx.enter_context(tc.tile_pool(name="z", bufs=4))

        w2 = bnd.tile([128, 2, E], FP8, name="w2")
        ksbd = bnd.tile([128, 2, 8], BF16, name="ksbd")

        ksum_sb = []
        kvr_sb = []
        for c in (0, 1):
            ks = bnd.tile([128, 1], F32, tag=f"ksum{c}")
            nc.vector.tensor_scalar(ks[:], kvp[c][:, 256:257], 1.0 / L, None, OP.mult)
            ksum_sb.append(ks)
            kr = bnd.tile([128, 256], BF16, tag=f"kvr{c}")
            nc.scalar.mul(kr[:], kvp[c][:, 0:256], 1.0 / L)
            kvr_sb.append(kr)
            nc.vector.tensor_scalar(
                ksbd[:, c, :], mh8[:, c, :], ks[:], None, OP.mult
            )

        ctx_kv.close()
        ctx_bd = ctx.enter_context(ExitStack())
        ps_bd = ctx_bd.enter_context(tc.tile_pool(name="ps_bd", bufs=1, space="PSUM"))
        ps_zi = ctx_bd.enter_context(tc.tile_pool(name="ps_zi", bufs=4, space="PSUM"))

        def emit_zi(gb):
            zi4 = ps_zi.tile([128, GRP], F32, tag="zi4")
            nc.tensor.matmul(
                zi4[:], mask1[:], one512[:], start=True, stop=False,
                skip_group_check=True,
            )
            for j in range(4):
                g = 4 * gb + j
                for c in (0, 1):
                    nc.tensor.matmul(
                        zi4[32 * j : 32 * j + 8, :], ksbd[:, c, :],
                        qfull[:, c, g, :], start=False,
                        stop=(j == 3 and c == 1),
                        tile_position=(0, 32 * j), skip_group_check=True,
                    )
            zln = zp.tile([128, GRP], F32, tag="zln")
            nc.scalar.activation(zln[:], zi4[:], AF.Ln)
            zs4 = zp.tile([128, GRP], BF16, tag="zs4", name=f"zs4_{gb}")
            nc.scalar.activation(zs4[:], zln[:], AF.Exp, scale=-1.0)
            return zs4

        zs_all = [emit_zi(0)]

        # transpose KV_raw/L: kvrT[b][e_loc, d_glob]
        kvrT = [bnd.tile([128, 256], BF16, tag=f"kvrT{b}", name=f"kvrT{b}") for b in (0, 1)]
        for b in (0, 1):
            for c in (0, 1):
                t_ps = ps_bd.tile([128, 128], BF16, tag="tp")
                nc.tensor.transpose(
                    t_ps[:], kvr_sb[c][:, 128 * b : 128 * (b + 1)], ident[:]
                )
                nc.scalar.copy(kvrT[b][:, 128 * c : 128 * (c + 1)], t_ps[:])

        zs_all.append(emit_zi(1))

        kvbds = []
        for c in (0, 1):
            csl = slice(128 * c, 128 * (c + 1))
            kvt_ps = ps_bd.tile([128, E], F32, tag="kvt")
            nc.tensor.matmul(
                kvt_ps[:], kvrT[0][:, csl], wv[:, 0, :], start=True, stop=False
            )
            nc.tensor.matmul(
                kvt_ps[:], kvrT[1][:, csl], wv[:, 1, :], start=False, stop=True
            )
            s_t = bnd.tile([128, 128], BF16, tag=f"s{c}")
            nc.vector.scalar_tensor_tensor(
                s_t[:], bvb[:, csl], ksum_sb[c][:], kvt_ps[:, csl],
                OP.mult, OP.add,
            )
            kvbd = bnd.tile([128, 128], BF16, tag=f"kvbd{c}")
            nc.vector.tensor_tensor(kvbd[:], s_t[:], mbd[:], OP.mult)
            kvbds.append(kvbd)

        zs_all.append(emit_zi(2))

        for c in (0, 1):
            tb_ps = ps_bd.tile([128, 128], BF16, tag="tb")
            nc.tensor.transpose(tb_ps[:], kvbds[c][:], ident[:])
            kvbdT = bnd.tile([128, 128], BF16, tag=f"kvbdT{c}")
            nc.scalar.copy(kvbdT[:], tb_ps[:])
            w2_ps = ps_bd.tile([128, E], F32, tag="w2p")
            nc.tensor.matmul(
                w2_ps[:], kvbdT[:], wm[:, c, :], start=True, stop=True
            )
            nc.scalar.mul(w2[:, c, :], w2_ps[:], SW2)

        zs_all.append(emit_zi(3))

        ctx_bd.close()

        # ================= phase B: ze -> qfts -> out ===================
        qfsp = ctx.enter_context(tc.tile_pool(name="qfs", bufs=3))
        osp = ctx.enter_context(tc.tile_pool(name="osb", bufs=6))
        ps_ze = ctx.enter_context(tc.tile_pool(name="ps_ze", bufs=3, space="PSUM"))
        ps_o = ctx.enter_context(tc.tile_pool(name="ps_o", bufs=2, space="PSUM"))

        def emit_ze(g):
            gb, j = g // 4, g % 4
            ze_ps = ps_ze.tile([128, 2, GRP], F32, tag="ze")
            for c in (0, 1):
                nc.tensor.matmul(
                    ze_ps[:, c, :], em8[32 * j : 32 * j + 8, c, :],
                    zs_all[gb][32 * j : 32 * j + 8, :], start=True, stop=True,
                    tile_position=(32 * j, 0),
                )
            return ze_ps

        zes = [emit_ze(0), emit_ze(1), emit_ze(2)]
        for g in range(NG):
            qfts = qfsp.tile([128, 2, GRP], FP8, tag="qfts")
            nc.vector.tensor_tensor(
                qfts[:], qfull[:, :, g, :], zes[g % 3][:], OP.mult
            )
            if g < NG - 3:
                zes[g % 3] = emit_ze(g + 3)
            o_sb = osp.tile([128, 4 * E], F16, tag="osb")
            for h in (0, 1):
                o_ps = ps_o.tile([128, 2, E], F32, tag="o")
                for t in (0, 1):
                    nc.tensor.matmul(
                        o_ps[:, t, :],
                        qfts[:, :, 256 * h + 128 * t : 256 * h + 128 * t + 128],
                        w2[:],
                        start=True, stop=True, perf_mode=DR,
                    )
                hs = slice(2 * E * h, 2 * E * (h + 1))
                nc.scalar.mul(
                    o_sb[:, hs], o_ps[:].rearrange("p a b -> p (a b)"), OUT_SCALE
                )
                nc.sync.dma_start(out_h[g, :, hs], o_sb[:, hs])

    if fix_waits:
        _fix_waits(nc)
    return nc


_WAIT_EXEMPT = {"InstEventSemaphore", "InstUnconditionalBranch", "InstISA"}


def _fix_waits(nc):
    """TPB ISA structs hold limited sem-wait slots; move excess waits onto
    sequencer EventSemaphore instructions inserted before the instruction."""
    n = 0
    for fn in nc.m.functions:
        for blk in fn.blocks:
            il = blk.instructions
            new = []
            changed = False
            for inst in il:
                tname = type(inst).__name__
                if tname not in _WAIT_EXEMPT:
                    limit = 0 if tname == "InstDmaTransposeAnt" else 1
                    si = inst.sync_info
                    waits = list(si.on_wait) if si is not None and si.on_wait else []
                    if len(waits) > limit:
                        move, keep = waits[: len(waits) - limit], waits[len(waits) - limit :]
                        for w in move:
                            es = mybir.InstEventSemaphore(
                                name=f"wait_fence_{n}", ins=[], outs=[],
                                engine=inst.engine,
                            )
                            es.sync_info = mybir.SyncInfo(on_wait=[w], on_update=[])
                            new.append(es)
                            n += 1
                        inst.sync_info = mybir.SyncInfo(
                            on_wait=keep,
                            on_update=list(si.on_update) if si.on_update else [],
                        )
                        changed = True
                new.append(inst)
            if changed:
                blk.instructions = new


_NC = None


def _get_nc():
    global _NC
    if _NC is None:
        _NC = build_nc()
    return _NC


def _host_consts(inputs):
    import ml_dtypes  # noqa

    fp8 = mybir.dt.np(FP8)
    bf16 = ml_dtypes.bfloat16
    Wq, Wk, Wv, Wm = (np.asarray(inputs[n], np.float32) for n in ("Wq", "Wk", "Wv", "Wm"))
    bq, bk, bv = (np.asarray(inputs[n], np.float32) for n in ("bq", "bk", "bv"))

    def wlayout(W, dt):
        # [p, i, e_out] = W.T[(i*128+p), e_out]
        return np.ascontiguousarray(
            W.T.reshape(2, 128, E).transpose(1, 0, 2)
        ).astype(dt)

    consts = {
        "wq8": wlayout(Wq, fp8),
        "wk8": wlayout(Wk, fp8),
        "wv16": wlayout(Wv, bf16),
        "wm16": wlayout(Wm, bf16),
        "bq2": np.ascontiguousarray(bq.reshape(2, 128).T),
        "bk2": np.ascontiguousarray(np.tile(bk + 1.0, 2)[None, :]).astype(bf16),
        "ones1": np.ones((1, 128), bf16),
        "neg1": np.full((128, 1), -1.0, np.float32),
        "bq1p": np.ascontiguousarray(bq.reshape(2, 128).T) + 1.0,
        "one512": np.ones((1, 512), bf16),
        "mask1": ((np.arange(128)[None, :] % 32) >= 8).astype(bf16),
        "bvb": np.ascontiguousarray(np.broadcast_to(bv, (128, E))),
    }
    p = np.arange(128)
    consts["mbd"] = ((p[:, None] // 32) == (p[None, :] // 32)).astype(bf16)
    mh8 = np.zeros((128, 2, 8), np.float32)
    for c in (0, 1):
        mh8[:, c, :] = (np.arange(8)[None, :] == (4 * c + p[:, None] // 32)) / TS
    consts["mh8"] = mh8.astype(bf16)
    em8r = np.zeros((128, 2, 128), np.float32)
    for c in (0, 1):
        pm = p[:, None] % 32
        em8r[:, c, :] = (pm < 8) & ((p[None, :] // 32) == (pm - 4 * c))
    consts["em8r"] = em8r.astype(bf16)
    consts["ident16"] = np.eye(128, dtype=bf16)
    return consts


def _prep_inputs(q, k, v):
    import ml_dtypes
    """q,k,v: [L, E] fp32 -> qT8/kT8 [NG,128,1024], vx8 [NG,128,1028] fp8."""
    fp8 = mybir.dt.np(FP8)

    def tlayout(x):
        # xT[p, i, l] = x[l, i*128+p]; split l into quads
        xT = x.T.reshape(2, 128, L).transpose(1, 0, 2)  # [p, i, l]
        xq = xT.reshape(128, 2, NG, GRP).transpose(2, 0, 1, 3)  # [u, p, i, l']
        return np.ascontiguousarray(xq.reshape(NG, 128, 2 * GRP)).astype(fp8)

    vx = np.ones((NG, 128, 4, 257), np.float32)
    vn = v.reshape(NG, 4, 128, 256).transpose(0, 2, 1, 3)  # [u, p, t, e]
    vx[:, :, :, 0:256] = vn
    return {
        "qT8": tlayout(q),
        "kT8": tlayout(k),
        "vx16": np.ascontiguousarray(vx.reshape(NG, 128, 4 * 257)).astype(ml_dtypes.bfloat16),
    }


def _make_in_maps(inputs):
    consts = _host_consts(inputs)
    q = np.asarray(inputs["q"], np.float32)
    k = np.asarray(inputs["k"], np.float32)
    v = np.asarray(inputs["v"], np.float32)

    in_maps = []
    for b in range(NCORES):
        m = dict(consts)
        m.update(_prep_inputs(q[b], k[b], v[b]))
        in_maps.append(m)
    return in_maps


def _unpack_out(res_out):
    # [NG, 128, 4*E] fp16 -> [L, E] fp32; row = 512g + 128t + p
    a = np.asarray(res_out).reshape(NG, 128, 4, E).transpose(0, 2, 1, 3)
    return a.reshape(L, E).astype(np.float32)


def kernel(**inputs):
    nc = _get_nc()
    res = run_bass_kernel_spmd(nc, _make_in_maps(inputs), list(range(NCORES)))
    out = np.stack([_unpack_out(res.results[b]["out"]) for b in range(NCORES)])
    return out.astype(np.float32)


def kernel_traced(**inputs):
    nc = _get_nc()
    res = run_bass_kernel_spmd(
        nc, _make_in_maps(inputs), list(range(NCORES)), trace=True
    )
    out = np.stack([_unpack_out(res.results[b]["out"]) for b in range(NCORES)])
    return out.astype(np.float32), res
